# revision 11
# baseline (speedup 1.0000x reference)
"""Trainium2 Bass kernel for nn_BasicClassifier (spiking conv classifier).

Sharding: pure data parallelism — batch 256 is split 32 samples per core
across 8 NeuronCores; params are replicated (tiny).

Per-core design. The T=1000 LIF scan is sequential, so the serial chain of
per-tick DVE ops paces everything. v2 cuts the per-op cost ~2.3x vs the
fp32/PSUM baseline by running the fused LIF op in the DVE's 2X_1PORT perf
mode: all operands are fp16, packed stride-1 in SBUF, and the op carries a
hand-written 2x uop program (element 0 in pipeline stages 0-3, element 1 in
stages 4-7 via the SRC_*_HI ports, results on WR0_LO/WR0_HI).

Memory plan per 16-tick block:
  - PSUM C tile [128, 4*512] fp32: banks 0-2 conv (bf16 hi/lo K-stacked
    GEMMs, ones row folds conv_b), bank 3 = fc bias prefill + fc result.
  - GpSimd copies the whole C tile to a tick-major fp16 SBUF tile (the DVE
    reads 128-col contiguous fp16 slices — 2x eligible).
  - state ring: fp16 [128, 16*128] x2. Per tick ONE fused DVE op:
    m' = (m*0.9 + c) - (m > 1).
  - spikes: ACT Sign over the fp16 ring -> sigma bf16; fc is a single-bf16
    halved-weight GEMM (3 matmuls) into bank 3 of the C tile 3 blocks ahead
    (the deep skew frees the copy from same-block dependencies).
  - mem2 history: fp16 DMA from ring cols 96:128 to DRAM; host sums.
"""

import os
import sys

for _p in ("/opt/trn_rl_repo", "/opt/pypackages"):
    if _p not in sys.path:
        sys.path.insert(0, _p)

import numpy as np

import concourse.bacc as bacc
import concourse.mybir as mybir
import concourse.tile as tile
import concourse.dve_ops as dve_ops
from concourse.dve_spec import Spec, Src0, Src1, C0, C1, lower
from concourse.dve_uop import (
    DveOpSpec, UopConfig, UopDpConfig, AluOp, AluInp, InpSel, OutSel, OutPath,
    DelayInp, Trigger, ENABLE, DISABLE, N_STAGES,
)
from concourse.bass_utils import run_bass_kernel_spmd

F32 = mybir.dt.float32
F16 = mybir.dt.float16
BF16 = mybir.dt.bfloat16
ALU = mybir.AluOpType
AF = mybir.ActivationFunctionType
AX = mybir.AxisListType

N_CORES = 8
B_FULL, T_FULL, L_IN = 256, 1000, 30
BC = B_FULL // N_CORES      # 32 samples per core
CH, LO = 16, 24
F = CH * LO                 # 384 features
G = 3                       # feature groups of 128
J = 35                      # fc outputs
KX = L_IN + 1               # conv contraction rows (30 taps + ones row)
BLK = 16                    # ticks per block (N = 16*32 = 512 = 1 PSUM bank)
LEAD = 2                    # C-tile pipeline lead in blocks (PSUM = 2 live tiles)
SKEW = LEAD * BLK           # layer-2 lag: spikes at tick t drive m2 at t+SKEW
WIN = 160                   # ticks per x-window DMA (multiple of BLK)
BETA, THR = 0.9, 1.0

TRACE = bool(int(os.environ.get("KERNEL_TRACE", "0")))
PERF_MAX = int(os.environ.get("KERNEL_PERF", "1"))
LAST_RESULTS = None

_LIF_OP = None

PREV = AluInp.PREV_ALU_OUT
SRC_DONE = (Trigger.SRC_TENSOR_DONE, Trigger.NONE, Trigger.NONE)


def _lif_2x_uop():
    """Hand-written 2X_1PORT program for m' = (m*C0 + c) - (m > C1).

    Per cycle the engine feeds two consecutive fp16 elements (one 32-bit
    word): SRC_0/SRC_1 = element 0, SRC_0_HI/SRC_1_HI = element 1. Stages
    0-3 compute element 0, stages 4-7 element 1; results ride delay lane 0
    (lo) and the final ALU out (hi).

    lanes: 0=SRC_0, 1=C0, 2=SRC_1, 3=C1, 4=SRC_0_HI, 5=SRC_1_HI
    """
    n = N_STAGES["v3"]
    dp = [UopDpConfig() for _ in range(n)]
    live = [
        (0, 1, 2, 3, 4, 5),   # s0
        (0, 1, 2, 3, 4, 5),   # s1
        (1, 3, 4, 5),         # s2 (lane0 captured here)
        (0, 1, 3, 4, 5),      # s3
        (3, 4, 5),            # s4 (lane0 captured here)
        (0, 3, 4, 5),         # s5
        (0, 3),               # s6 (lane4 captured here)
        (0, 4),               # s7
    ]
    for st in range(n):
        dp[st].pass_through_delay(*live[st])
    D = lambda i: AluInp(int(AluInp.PREV_DELAY_0) + i)
    dp[0].enable_alu(AluOp.MULTIPLY, D(0), D(1))          # m_lo * beta
    dp[1].enable_alu(AluOp.ADD, PREV, D(2))               # + c_lo
    dp[2].enable_alu(AluOp.IS_LT, D(3), D(0))             # thr < m_lo
    dp[2].enable_delay_from_src(DelayInp.PREV_ALU_OUT, 0)  # lane0 <- t2_lo
    dp[3].enable_alu(AluOp.SUBTRACT, D(0), PREV)          # res_lo
    dp[4].enable_alu(AluOp.MULTIPLY, D(4), D(1))          # m_hi * beta
    dp[4].enable_delay_from_src(DelayInp.PREV_ALU_OUT, 0)  # lane0 <- res_lo
    dp[5].enable_alu(AluOp.ADD, PREV, D(5))               # + c_hi
    dp[6].enable_alu(AluOp.IS_LT, D(3), D(4))             # thr < m_hi
    dp[6].enable_delay_from_src(DelayInp.PREV_ALU_OUT, 4)  # lane4 <- t2_hi
    dp[7].enable_alu(AluOp.SUBTRACT, D(4), PREV)          # res_hi

    inp = [InpSel.ZERO] * 8
    inp_enable = [DISABLE] * 8
    for slot, sel in ((1, InpSel.SRC_0), (2, InpSel.CONST_0), (3, InpSel.SRC_1),
                      (4, InpSel.CONST_1), (5, InpSel.SRC_0_HI),
                      (6, InpSel.SRC_1_HI)):
        inp[slot], inp_enable[slot] = sel, ENABLE
    out = {o: OutSel.ALU_OUT for o in OutPath}
    out_enable = {o: DISABLE for o in OutPath}
    out[OutPath.WR0_LO], out_enable[OutPath.WR0_LO] = OutSel.DELAY_0, ENABLE
    out[OutPath.WR0_HI], out_enable[OutPath.WR0_HI] = OutSel.ALU_OUT, ENABLE
    return UopConfig(
        datapath_config=dp,
        inp=inp,
        inp_enable=inp_enable,
        out=out,
        out_enable=out_enable,
        accum_enabled=DISABLE,
        require_inp0=1,
        require_inp1=1,
        trigger=SRC_DONE,
        next_uop=(0, 0, 0),
        repeat_count=0,
    )


def _get_lif_op():
    """Register the fused LIF-step op (with a 2x perf program) in the
    custom-DVE table (idempotent)."""
    global _LIF_OP
    if _LIF_OP is not None:
        return _LIF_OP
    name = "LIF_STEP_ANT59"
    for op in dve_ops.OPS:
        if op.name == name:
            _LIF_OP = op
            return op
    spec = Spec(
        body=(Src0 * C0 + Src1) - (Src0 > C1),
        reference=lambda in0, in1, s0, s1, imm2: (
            (in0.astype(np.float32) * np.float32(s0)
             + in1.reshape(in0.shape))
            - (in0 > s1).astype(np.float32)
        ).astype(np.float32),
    )
    row = dve_ops._CUSTOM_DVE_ROW_BASE + len(dve_ops.OPS)
    assert row < 0x20
    dve_ops._SUB_OPCODE_FOR_NAME[name] = row
    compiled = DveOpSpec(
        name=name, opcode=row, uops=lower(spec, ver="v3"), rd1_en=True,
        uops_2x=[_lif_2x_uop()], perf_max=1,
    )
    compiled.validate("v3")
    shas = {"v3": compiled.sha("v3")}
    op = dve_ops.DveOp(name, spec, subdim=False, uops_sha=shas)
    dve_ops.OPS.append(op)
    dve_ops.CUSTOM_DVE_SPECS[name] = spec
    # compile() path must return the spec WITH the 2x program: pre-seed the
    # memo that both _custom_dve and dve_table_for_ops consult.
    dve_ops._COMPILE_CACHE[(name, "v3")] = compiled
    _LIF_OP = op
    return op


def _build_nc(T):
    """Build the per-core Bass program (SPMD: same program on every core)."""
    lif = _get_lif_op()
    ticks = T + SKEW                       # DVE ticks 0..T+SKEW-1
    nblk = -(-ticks // BLK)
    pad_ticks = nblk * BLK
    windows = -(-pad_ticks // WIN)
    xt_cols = windows * WIN * BC
    NB = BLK * BC                          # 512: one PSUM bank of f32

    nc = bacc.Bacc("TRN2", target_bir_lowering=False)

    KS = 3 * KX                            # stacked conv K: [xh; xl; xh]
    xts_d = nc.dram_tensor("xts", [KS, xt_cols], BF16, kind="ExternalInput")
    wes_d = nc.dram_tensor("wes", [KS, F], BF16, kind="ExternalInput")
    fch_d = nc.dram_tensor("fch", [128, G * J], BF16, kind="ExternalInput")
    brs_d = nc.dram_tensor("brs", [2, 128], BF16, kind="ExternalInput")
    ones_d = nc.dram_tensor("ones", [2, NB], BF16, kind="ExternalInput")
    hist_d = nc.dram_tensor("hist", [J, BC * T], F16, kind="ExternalOutput")

    with tile.TileContext(nc) as tc:
        with (
            tc.tile_pool(name="konst", bufs=1) as kp,
            tc.tile_pool(name="ring", bufs=1) as rp,
            tc.tile_pool(name="sig", bufs=2) as sgp,
            tc.tile_pool(name="xwin", bufs=2) as xp,
            tc.tile_pool(name="csb", bufs=3) as cbp,
            tc.tile_pool(name="cpsum", bufs=2, space="PSUM") as cp,
        ):
            # constants -> SBUF
            wes = kp.tile([KS, F], BF16, tag="wes")
            fch = kp.tile([128, G * J], BF16, tag="fch")
            brs = kp.tile([2, 128], BF16, tag="brs")
            ones = kp.tile([2, NB], BF16, tag="ones")
            for sb, dr in ((wes, wes_d), (fch, fch_d),
                           (brs, brs_d), (ones, ones_d)):
                nc.sync.dma_start(sb[:], dr[:])

            # state ring: 2 block-sized fp16 tiles of 16 slices each
            ringA = rp.tile([128, BLK * 128], F16, tag="ringA")
            ringB = rp.tile([128, BLK * 128], F16, tag="ringB")
            nc.vector.memset(ringA[:], 0.0)
            nc.vector.memset(ringB[:], 0.0)
            rings = (ringA, ringB)

            xts = {}      # window idx -> xt sbuf tile
            chs = {}      # block idx -> PSUM C tile [128, 4*512] fp32
            csbs = {}     # block idx -> SBUF fp16 C tile [128, 16*128]
            def load_window(w):
                if w < 0 or w >= windows or w in xts:
                    return
                ts = xp.tile([KS, WIN * BC], BF16, tag="xws")
                nc.sync.dma_start(ts[:], xts_d[:, w * WIN * BC:(w + 1) * WIN * BC])
                xts[w] = ts

            def ensure_psum(b):
                """Allocate block b's PSUM C tile; conv-fill banks 0-2 and
                prime bank 3 with the fc bias (zeros for b < LEAD)."""
                if b >= nblk or b in chs:
                    return
                ch = cp.tile([128, 4 * NB], F32, tag="ch")
                chs[b] = ch
                w = (b * BLK) // WIN
                base = (b * BLK - w * WIN) * BC
                for g in range(G):
                    nc.tensor.matmul(
                        out=ch[:, g * NB:(g + 1) * NB],
                        lhsT=wes[:, g * 128:(g + 1) * 128],
                        rhs=xts[w][:, base:base + NB],
                        start=True, stop=True,
                    )
                if b >= LEAD:
                    nc.tensor.matmul(
                        out=ch[:, G * NB:4 * NB],
                        lhsT=brs[:, :], rhs=ones[:, :],
                        start=True, stop=False,
                        skip_group_check=True,
                    )
                else:
                    nc.vector.memset(ch[:, G * NB:4 * NB], 0.0)

            def conv_copy(b):
                """PSUM banks 0-2 of C tile b -> tick-major fp16 SBUF tile
                (ACT; GpSimd cannot read PSUM). psum col = g*512 + t*32 + c ;
                fp16 col = t*128 + g*32 + c. No fc dependency: runs early."""
                if b >= nblk or b in csbs:
                    return
                cs = cbp.tile([128, BLK * 128], F16, tag="cs")
                csbs[b] = cs
                src = chs[b][:, 0:G * NB].rearrange("p (g t c) -> p t g c", g=G, c=BC)
                dst = cs[:].rearrange("p (t g c) -> p t g c", g=4, c=BC)[:, :, 0:G, :]
                nc.scalar.activation(out=dst, in_=src, func=AF.Copy)

            def fc_copy(b):
                """PSUM bank 3 (fc drive) of C tile b -> fp16 cols 96:128
                (ACT; short chain after the same-block fc matmuls)."""
                if b >= nblk:
                    return
                src = chs[b][:, G * NB:4 * NB].rearrange("p (t c) -> p t c", c=BC)
                dst = csbs[b][:].rearrange("p (t g c) -> p t g c", g=4, c=BC)[:, :, G, :]
                nc.scalar.activation(out=dst, in_=src, func=AF.Copy)

            def spikes_and_fc(b):
                """After block b's ticks: s = (m1 > 1) in {0,1} per group
                (GpSimd, off the PSUM-capable ACT engine), each feeding its
                fc matmul into bank 3 of C tile b+LEAD."""
                if b < 0 or b + LEAD >= nblk:
                    return
                ring4 = rings[b % 2][:].rearrange("p (t g c) -> p t g c", g=4, c=BC)
                sg = sgp.tile([128, G * NB], BF16, tag="sg")
                for g in range(G):
                    nc.gpsimd.tensor_scalar(
                        out=sg[:, g * NB:(g + 1) * NB].rearrange("p (t c) -> p t c", c=BC),
                        in0=ring4[:, :, g, :], scalar1=THR, scalar2=None,
                        op0=ALU.is_gt,
                    )
                    nc.tensor.matmul(
                        out=chs[b + LEAD][0:J, G * NB:4 * NB],
                        lhsT=fch[:, g * J:(g + 1) * J],
                        rhs=sg[:, g * NB:(g + 1) * NB],
                        start=False, stop=(g == G - 1),
                        skip_group_check=True,
                    )

            def hist_dma(b):
                """mem2 of DVE-tick block b = m2 ticks [16b-SKEW, ...):
                DMA straight from the fp16 ring to DRAM (host sums)."""
                t0 = b * BLK - SKEW
                if t0 < 0:
                    return
                n = min(BLK, T - t0)
                if n <= 0:
                    return
                ring3 = rings[b % 2][:].rearrange("p (t c) -> p t c", c=128)
                nc.sync.dma_start(
                    hist_d[:, t0 * BC:(t0 + n) * BC],
                    ring3[0:J, 0:n, G * BC:128],
                )

            # prologue: C pipeline primed LEAD blocks deep
            load_window(0)
            load_window(1)
            ensure_psum(0)
            ensure_psum(1)
            conv_copy(0)
            fc_copy(0)

            for t in range(ticks):
                b, lo = divmod(t, BLK)
                if lo == 0:
                    load_window(((b + 4) * BLK) // WIN)
                    ensure_psum(b + LEAD)
                    conv_copy(b + 1)
                    spikes_and_fc(b - 1)
                    fc_copy(b + 1)
                    hist_dma(b - 1)
                ring = rings[b % 2]
                prev = rings[(b - 1) % 2] if lo == 0 else ring
                plo = (lo - 1) % BLK

                inst = nc.vector._custom_dve(
                    lif,
                    out=ring[:, lo * 128:(lo + 1) * 128],
                    in0=prev[:, plo * 128:(plo + 1) * 128],
                    in1=csbs[b][:, lo * 128:(lo + 1) * 128],
                    s0=BETA, s1=THR,
                )
                inst.ins.perf_max = PERF_MAX

            # epilogue: the last block's mem2 history
            hist_dma(nblk - 1)

    nc.compile()
    return nc


def _bf16_split(a):
    import ml_dtypes
    hi = a.astype(ml_dtypes.bfloat16)
    lo = (a - hi.astype(np.float32)).astype(ml_dtypes.bfloat16)
    return hi, lo


def _host_prep(x, conv_w, conv_b, fc_w, fc_b, T):
    """Build per-core input maps (numpy only)."""
    import ml_dtypes
    ticks = T + SKEW
    nblk = -(-ticks // BLK)
    windows = -(-(nblk * BLK) // WIN)
    xt_ticks = windows * WIN

    wexp = np.zeros((KX, F), np.float32)
    for c in range(CH):
        for l in range(LO):
            wexp[l:l + 7, c * LO + l] = conv_w[c, 0, :]
        wexp[L_IN, c * LO:(c + 1) * LO] = conv_b[c]
    weh, wel = _bf16_split(wexp)
    wes = np.concatenate([weh, weh, wel], axis=0)  # K-stacked [93, F]

    # spikes s = (m > 1) in {0,1} (GpSimd is_gt): c2 = fc_w @ s + b
    fcwt = np.zeros((128, G * J), np.float32)
    for g in range(G):
        fcwt[:, g * J:(g + 1) * J] = fc_w[:, g * 128:(g + 1) * 128].T
    fch = fcwt.astype(ml_dtypes.bfloat16)
    brow = np.zeros((1, 128), np.float32)
    brow[0, :J] = fc_b
    brh, brl = _bf16_split(brow)
    brs = np.concatenate([brh, brl], axis=0)       # [2, 128]

    ones = np.ones((2, BLK * BC), ml_dtypes.bfloat16)

    in_maps = []
    B = x.shape[0]
    n_cores = B // BC
    for core in range(n_cores):
        xc = x[core * BC:(core + 1) * BC]          # [BC, T, L]
        xt = np.zeros((KX, xt_ticks, BC), np.float32)
        xt[:L_IN, :T, :] = xc.transpose(2, 1, 0)
        xt[L_IN, :T, :] = 1.0
        xt = xt.reshape(KX, xt_ticks * BC)
        xth, xtl = _bf16_split(xt)
        xstk = np.concatenate([xth, xtl, xth], axis=0)  # [93, cols]
        in_maps.append({
            "xts": xstk, "wes": wes, "fch": fch,
            "brs": brs, "ones": ones,
        })
    return in_maps


def _install_trace_hook():
    """Wire up the axon NTFF profiling hook (absent from this image)."""
    import types

    if "antenv.axon_hooks" in sys.modules:
        return True
    try:
        if "/root/.axon_site" not in sys.path:
            sys.path.insert(0, "/root/.axon_site")
        from trn_agent_boot.trn_boot import _ntff_profile_via_ctypes

        hook = _ntff_profile_via_ctypes("/opt/axon/libaxon_pjrt.so")
        if hook is None:
            return False
        mod = types.ModuleType("antenv.axon_hooks")
        mod.get_axon_ntff_profile_hook = lambda: hook
        sys.modules["antenv.axon_hooks"] = mod
        import concourse.bass_utils as bu

        bu.upload_artifacts = lambda tmpdir: str(tmpdir)
        return True
    except Exception as e:  # profiling is optional
        print(f"trace hook install failed: {e}", file=sys.stderr)
        return False


def run_cores(x, conv_w, conv_b, fc_w, fc_b, T=None):
    """Run the Bass kernel on len(batch)/32 cores; returns [B, 35] output."""
    global LAST_RESULTS
    T = T if T is not None else x.shape[1]
    trace = TRACE and _install_trace_hook()
    nc = _build_nc(T)
    in_maps = _host_prep(x, conv_w, conv_b, fc_w, fc_b, T)
    res = run_bass_kernel_spmd(
        nc, in_maps, core_ids=list(range(len(in_maps))), trace=trace,
    )
    LAST_RESULTS = res
    outs = []
    for i in range(len(in_maps)):
        hv = np.asarray(res.results[i]["hist"], dtype=np.float32)
        m2 = hv.reshape(J, T, BC)                  # [J, t, sample]
        outs.append((m2.sum(axis=1) / np.float32(T)).T.astype(np.float32))
    return np.concatenate(outs, axis=0)


def kernel(x, conv_w, conv_b, fc_w, fc_b):
    return run_cores(
        np.asarray(x, np.float32), np.asarray(conv_w, np.float32),
        np.asarray(conv_b, np.float32), np.asarray(fc_w, np.float32),
        np.asarray(fc_b, np.float32),
    )


# revision 14
# speedup vs baseline: 4.8934x; 4.8934x over previous
"""Trainium2 Bass kernel for nn_BasicClassifier (spiking conv classifier).

Sharding: pure data parallelism — batch 256 is split 32 samples per core
across 8 NeuronCores; params are replicated (tiny).

Per-core design. The T=1000 LIF scan is sequential, so the serial chain of
per-tick DVE ops paces everything. v2 cuts the per-op cost ~2.3x vs the
fp32/PSUM baseline by running the fused LIF op in the DVE's 2X_1PORT perf
mode: all operands are fp16, packed stride-1 in SBUF, and the op carries a
hand-written 2x uop program (element 0 in pipeline stages 0-3, element 1 in
stages 4-7 via the SRC_*_HI ports, results on WR0_LO/WR0_HI).

Memory plan per 16-tick block:
  - PSUM C tile [128, 4*512] fp32: banks 0-2 conv (bf16 hi/lo K-stacked
    GEMMs, ones row folds conv_b), bank 3 = fc bias prefill + fc result.
  - GpSimd copies the whole C tile to a tick-major fp16 SBUF tile (the DVE
    reads 128-col contiguous fp16 slices — 2x eligible).
  - state ring: fp16 [128, 16*128] x2. Per tick ONE fused DVE op:
    m' = (m*0.9 + c) - (m > 1).
  - spikes: ACT Sign over the fp16 ring -> sigma bf16; fc is a single-bf16
    halved-weight GEMM (3 matmuls) into bank 3 of the C tile 3 blocks ahead
    (the deep skew frees the copy from same-block dependencies).
  - mem2 history: fp16 DMA from ring cols 96:128 to DRAM; host sums.
"""

import os
import sys

for _p in ("/opt/trn_rl_repo", "/opt/pypackages"):
    if _p not in sys.path:
        sys.path.insert(0, _p)

import numpy as np

import concourse.bacc as bacc
import concourse.mybir as mybir
import concourse.tile as tile
import concourse.dve_ops as dve_ops
from concourse.dve_spec import Spec, Src0, Src1, C0, C1, lower
from concourse.dve_uop import (
    DveOpSpec, UopConfig, UopDpConfig, AluOp, AluInp, InpSel, OutSel, OutPath,
    DelayInp, Trigger, ENABLE, DISABLE, N_STAGES,
)
from concourse.bass_utils import run_bass_kernel_spmd

F32 = mybir.dt.float32
F16 = mybir.dt.float16
BF16 = mybir.dt.bfloat16
ALU = mybir.AluOpType
AF = mybir.ActivationFunctionType
AX = mybir.AxisListType

N_CORES = 8
B_FULL, T_FULL, L_IN = 256, 1000, 30
BC = B_FULL // N_CORES      # 32 samples per core
CH, LO = 16, 24
F = CH * LO                 # 384 features
G = 3                       # feature groups of 128
J = 35                      # fc outputs
KX = L_IN + 1               # conv contraction rows (30 taps + ones row)
BLK = 16                    # ticks per block (N = 16*32 = 512 = 1 PSUM bank)
LEAD = 2                    # C-tile pipeline lead in blocks (PSUM = 2 live tiles)
SKEW = LEAD * BLK           # layer-2 lag: spikes at tick t drive m2 at t+SKEW
WIN = 160                   # ticks per x-window DMA (multiple of BLK)
BETA, THR = 0.9, 1.0

TRACE = bool(int(os.environ.get("KERNEL_TRACE", "0")))
PERF_MAX = int(os.environ.get("KERNEL_PERF", "1"))
LAST_RESULTS = None

_LIF_OP = None

PREV = AluInp.PREV_ALU_OUT
SRC_DONE = (Trigger.SRC_TENSOR_DONE, Trigger.NONE, Trigger.NONE)


def _lif_2x_uop():
    """Hand-written 2X_1PORT program for m' = (m*C0 + c) - (m > C1).

    Per cycle the engine feeds two consecutive fp16 elements (one 32-bit
    word): SRC_0/SRC_1 = element 0, SRC_0_HI/SRC_1_HI = element 1. Stages
    0-3 compute element 0, stages 4-7 element 1; results ride delay lane 0
    (lo) and the final ALU out (hi).

    lanes: 0=SRC_0, 1=C0, 2=SRC_1, 3=C1, 4=SRC_0_HI, 5=SRC_1_HI
    """
    n = N_STAGES["v3"]
    dp = [UopDpConfig() for _ in range(n)]
    live = [
        (0, 1, 2, 3, 4, 5),   # s0
        (0, 1, 2, 3, 4, 5),   # s1
        (1, 3, 4, 5),         # s2 (lane0 captured here)
        (0, 1, 3, 4, 5),      # s3
        (3, 4, 5),            # s4 (lane0 captured here)
        (0, 3, 4, 5),         # s5
        (0, 3),               # s6 (lane4 captured here)
        (0, 4),               # s7
    ]
    for st in range(n):
        dp[st].pass_through_delay(*live[st])
    D = lambda i: AluInp(int(AluInp.PREV_DELAY_0) + i)
    dp[0].enable_alu(AluOp.MULTIPLY, D(0), D(1))          # m_lo * beta
    dp[1].enable_alu(AluOp.ADD, PREV, D(2))               # + c_lo
    dp[2].enable_alu(AluOp.IS_LT, D(3), D(0))             # thr < m_lo
    dp[2].enable_delay_from_src(DelayInp.PREV_ALU_OUT, 0)  # lane0 <- t2_lo
    dp[3].enable_alu(AluOp.SUBTRACT, D(0), PREV)          # res_lo
    dp[4].enable_alu(AluOp.MULTIPLY, D(4), D(1))          # m_hi * beta
    dp[4].enable_delay_from_src(DelayInp.PREV_ALU_OUT, 0)  # lane0 <- res_lo
    dp[5].enable_alu(AluOp.ADD, PREV, D(5))               # + c_hi
    dp[6].enable_alu(AluOp.IS_LT, D(3), D(4))             # thr < m_hi
    dp[6].enable_delay_from_src(DelayInp.PREV_ALU_OUT, 4)  # lane4 <- t2_hi
    dp[7].enable_alu(AluOp.SUBTRACT, D(4), PREV)          # res_hi

    inp = [InpSel.ZERO] * 8
    inp_enable = [DISABLE] * 8
    for slot, sel in ((1, InpSel.SRC_0), (2, InpSel.CONST_0), (3, InpSel.SRC_1),
                      (4, InpSel.CONST_1), (5, InpSel.SRC_0_HI),
                      (6, InpSel.SRC_1_HI)):
        inp[slot], inp_enable[slot] = sel, ENABLE
    out = {o: OutSel.ALU_OUT for o in OutPath}
    out_enable = {o: DISABLE for o in OutPath}
    out[OutPath.WR0_LO], out_enable[OutPath.WR0_LO] = OutSel.DELAY_0, ENABLE
    out[OutPath.WR0_HI], out_enable[OutPath.WR0_HI] = OutSel.ALU_OUT, ENABLE
    return UopConfig(
        datapath_config=dp,
        inp=inp,
        inp_enable=inp_enable,
        out=out,
        out_enable=out_enable,
        accum_enabled=DISABLE,
        require_inp0=1,
        require_inp1=1,
        trigger=SRC_DONE,
        next_uop=(0, 0, 0),
        repeat_count=0,
    )


def _get_lif_op():
    """Register the fused LIF-step op (with a 2x perf program) in the
    custom-DVE table (idempotent)."""
    global _LIF_OP
    if _LIF_OP is not None:
        return _LIF_OP
    name = "LIF_STEP_ANT59"
    for op in dve_ops.OPS:
        if op.name == name:
            _LIF_OP = op
            return op
    spec = Spec(
        body=(Src0 * C0 + Src1) - (Src0 > C1),
        reference=lambda in0, in1, s0, s1, imm2: (
            (in0.astype(np.float32) * np.float32(s0)
             + in1.reshape(in0.shape))
            - (in0 > s1).astype(np.float32)
        ).astype(np.float32),
    )
    row = dve_ops._CUSTOM_DVE_ROW_BASE + len(dve_ops.OPS)
    assert row < 0x20
    dve_ops._SUB_OPCODE_FOR_NAME[name] = row
    compiled = DveOpSpec(
        name=name, opcode=row, uops=lower(spec, ver="v3"), rd1_en=True,
        uops_2x=[_lif_2x_uop()], perf_max=1,
    )
    compiled.validate("v3")
    shas = {"v3": compiled.sha("v3")}
    op = dve_ops.DveOp(name, spec, subdim=False, uops_sha=shas)
    dve_ops.OPS.append(op)
    dve_ops.CUSTOM_DVE_SPECS[name] = spec
    # compile() path must return the spec WITH the 2x program: pre-seed the
    # memo that both _custom_dve and dve_table_for_ops consult.
    dve_ops._COMPILE_CACHE[(name, "v3")] = compiled
    _LIF_OP = op
    return op


def _build_nc(T):
    """Build the per-core Bass program (SPMD: same program on every core)."""
    lif = _get_lif_op()
    ticks = T + SKEW                       # DVE ticks 0..T+SKEW-1
    nblk = -(-ticks // BLK)
    pad_ticks = nblk * BLK
    windows = -(-pad_ticks // WIN)
    xt_cols = windows * WIN * BC
    NB = BLK * BC                          # 512: one PSUM bank of f32

    nc = bacc.Bacc("TRN2", target_bir_lowering=False)

    KS = 3 * KX                            # stacked conv K: [xh; xl; xh]
    xts_d = nc.dram_tensor("xts", [KS, xt_cols], BF16, kind="ExternalInput")
    wes_d = nc.dram_tensor("wes", [KS, F], BF16, kind="ExternalInput")
    fch_d = nc.dram_tensor("fch", [128, G * J], BF16, kind="ExternalInput")
    brs_d = nc.dram_tensor("brs", [2, 128], BF16, kind="ExternalInput")
    ones_d = nc.dram_tensor("ones", [2, NB], BF16, kind="ExternalInput")
    hist_d = nc.dram_tensor("hist", [J, BC * T], F16, kind="ExternalOutput")

    with tile.TileContext(nc) as tc:
        with (
            tc.tile_pool(name="konst", bufs=1) as kp,
            tc.tile_pool(name="ring", bufs=1) as rp,
            tc.tile_pool(name="sig", bufs=2) as sgp,
            tc.tile_pool(name="xwin", bufs=2) as xp,
            tc.tile_pool(name="csb", bufs=3) as cbp,
            tc.tile_pool(name="cpsum", bufs=2, space="PSUM") as cp,
        ):
            # constants -> SBUF
            wes = kp.tile([KS, F], BF16, tag="wes")
            fch = kp.tile([128, G * J], BF16, tag="fch")
            brs = kp.tile([2, 128], BF16, tag="brs")
            ones = kp.tile([2, NB], BF16, tag="ones")
            negthr = kp.tile([128, 1], F32, tag="negthr")
            nc.vector.memset(negthr[:], -THR)
            for sb, dr in ((wes, wes_d), (fch, fch_d),
                           (brs, brs_d), (ones, ones_d)):
                nc.sync.dma_start(sb[:], dr[:])

            # state ring: 2 block-sized fp16 tiles of 16 slices each
            ringA = rp.tile([128, BLK * 128], F16, tag="ringA")
            ringB = rp.tile([128, BLK * 128], F16, tag="ringB")
            nc.vector.memset(ringA[:], 0.0)
            nc.vector.memset(ringB[:], 0.0)
            rings = (ringA, ringB)

            xts = {}      # window idx -> xt sbuf tile
            chs = {}      # block idx -> PSUM C tile [128, 4*512] fp32
            csbs = {}     # block idx -> SBUF fp16 C tile [128, 16*128]
            def load_window(w):
                if w < 0 or w >= windows or w in xts:
                    return
                ts = xp.tile([KS, WIN * BC], BF16, tag="xws")
                nc.sync.dma_start(ts[:], xts_d[:, w * WIN * BC:(w + 1) * WIN * BC])
                xts[w] = ts

            def ensure_psum(b):
                """Allocate block b's PSUM C tile; conv-fill banks 0-2 and
                prime bank 3 with the fc bias (zeros for b < LEAD)."""
                if b >= nblk or b in chs:
                    return
                ch = cp.tile([128, 4 * NB], F32, tag="ch")
                chs[b] = ch
                w = (b * BLK) // WIN
                base = (b * BLK - w * WIN) * BC
                for g in range(G):
                    nc.tensor.matmul(
                        out=ch[:, g * NB:(g + 1) * NB],
                        lhsT=wes[:, g * 128:(g + 1) * 128],
                        rhs=xts[w][:, base:base + NB],
                        start=True, stop=True,
                    )
                if b >= LEAD:
                    nc.tensor.matmul(
                        out=ch[:, G * NB:4 * NB],
                        lhsT=brs[:, :], rhs=ones[:, :],
                        start=True, stop=False,
                        skip_group_check=True,
                    )
                else:
                    nc.vector.memset(ch[:, G * NB:4 * NB], 0.0)

            def conv_copy(b):
                """PSUM banks 0-2 of C tile b -> tick-major fp16 SBUF tile
                (ACT; GpSimd cannot read PSUM). psum col = g*512 + t*32 + c ;
                fp16 col = t*128 + g*32 + c. No fc dependency: runs early."""
                if b >= nblk or b in csbs:
                    return
                cs = cbp.tile([128, BLK * 128], F16, tag="cs")
                csbs[b] = cs
                src = chs[b][:, 0:G * NB].rearrange("p (g t c) -> p t g c", g=G, c=BC)
                dst = cs[:].rearrange("p (t g c) -> p t g c", g=4, c=BC)[:, :, 0:G, :]
                nc.scalar.activation(out=dst, in_=src, func=AF.Copy)

            def fc_copy(b):
                """PSUM bank 3 (fc drive) of C tile b -> fp16 cols 96:128
                (ACT; short chain after the same-block fc matmuls)."""
                if b >= nblk:
                    return
                src = chs[b][:, G * NB:4 * NB].rearrange("p (t c) -> p t c", c=BC)
                dst = csbs[b][:].rearrange("p (t g c) -> p t g c", g=4, c=BC)[:, :, G, :]
                nc.scalar.activation(out=dst, in_=src, func=AF.Copy)

            def spikes_and_fc(b):
                """After block b's ticks: sigma = Sign(m1 - 1) in {-1,0,1}
                (one ACT op over the fp16 ring; halved fc weights fold the
                (sigma+1)/2), then fc (3 bf16 matmuls) into C tile b+LEAD."""
                if b < 0 or b + LEAD >= nblk:
                    return
                ring4 = rings[b % 2][:].rearrange("p (t g c) -> p t g c", g=4, c=BC)
                sg = sgp.tile([128, G * NB], BF16, tag="sg")
                sg4 = sg[:].rearrange("p (g t c) -> p t g c", g=G, c=BC)
                nc.scalar.activation(
                    out=sg4, in_=ring4[:, :, 0:G, :],
                    func=AF.Sign, bias=negthr[:],
                )
                for g in range(G):
                    nc.tensor.matmul(
                        out=chs[b + LEAD][0:J, G * NB:4 * NB],
                        lhsT=fch[:, g * J:(g + 1) * J],
                        rhs=sg[:, g * NB:(g + 1) * NB],
                        start=False, stop=(g == G - 1),
                        skip_group_check=True,
                    )

            def hist_dma(b):
                """mem2 of DVE-tick block b = m2 ticks [16b-SKEW, ...):
                DMA straight from the fp16 ring to DRAM (host sums)."""
                t0 = b * BLK - SKEW
                if t0 < 0:
                    return
                n = min(BLK, T - t0)
                if n <= 0:
                    return
                ring3 = rings[b % 2][:].rearrange("p (t c) -> p t c", c=128)
                nc.sync.dma_start(
                    hist_d[:, t0 * BC:(t0 + n) * BC],
                    ring3[0:J, 0:n, G * BC:128],
                )

            # prologue: C pipeline primed LEAD blocks deep
            load_window(0)
            load_window(1)
            ensure_psum(0)
            ensure_psum(1)
            conv_copy(0)
            fc_copy(0)

            for t in range(ticks):
                b, lo = divmod(t, BLK)
                if lo == 0:
                    load_window(((b + 4) * BLK) // WIN)
                    ensure_psum(b + LEAD)
                    conv_copy(b + 1)
                    spikes_and_fc(b - 1)
                    fc_copy(b + 1)
                    hist_dma(b - 1)
                ring = rings[b % 2]
                prev = rings[(b - 1) % 2] if lo == 0 else ring
                plo = (lo - 1) % BLK

                inst = nc.vector._custom_dve(
                    lif,
                    out=ring[:, lo * 128:(lo + 1) * 128],
                    in0=prev[:, plo * 128:(plo + 1) * 128],
                    in1=csbs[b][:, lo * 128:(lo + 1) * 128],
                    s0=BETA, s1=THR,
                )
                inst.ins.perf_max = PERF_MAX

            # epilogue: the last block's mem2 history
            hist_dma(nblk - 1)

    nc.compile()
    return nc


def _bf16_split(a):
    import ml_dtypes
    hi = a.astype(ml_dtypes.bfloat16)
    lo = (a - hi.astype(np.float32)).astype(ml_dtypes.bfloat16)
    return hi, lo


def _host_prep(x, conv_w, conv_b, fc_w, fc_b, T):
    """Build per-core input maps (numpy only)."""
    import ml_dtypes
    ticks = T + SKEW
    nblk = -(-ticks // BLK)
    windows = -(-(nblk * BLK) // WIN)
    xt_ticks = windows * WIN

    wexp = np.zeros((KX, F), np.float32)
    for c in range(CH):
        for l in range(LO):
            wexp[l:l + 7, c * LO + l] = conv_w[c, 0, :]
        wexp[L_IN, c * LO:(c + 1) * LO] = conv_b[c]
    weh, wel = _bf16_split(wexp)
    wes = np.concatenate([weh, weh, wel], axis=0)  # K-stacked [93, F]

    # spike trick: s = (sigma+1)/2 with sigma = sign(m-1) in {-1,0,1}
    # c2 = fc_w @ s + b = (fc_w/2) @ sigma + (b + fc_w.sum/2)
    half = (fc_w * 0.5).astype(np.float32)
    fcwt = np.zeros((128, G * J), np.float32)
    for g in range(G):
        fcwt[:, g * J:(g + 1) * J] = half[:, g * 128:(g + 1) * 128].T
    fch = fcwt.astype(ml_dtypes.bfloat16)
    brow = np.zeros((1, 128), np.float32)
    brow[0, :J] = fc_b + half.sum(axis=1)
    brh, brl = _bf16_split(brow)
    brs = np.concatenate([brh, brl], axis=0)       # [2, 128]

    ones = np.ones((2, BLK * BC), ml_dtypes.bfloat16)

    in_maps = []
    B = x.shape[0]
    n_cores = B // BC
    for core in range(n_cores):
        xc = x[core * BC:(core + 1) * BC]          # [BC, T, L]
        xt = np.zeros((KX, xt_ticks, BC), np.float32)
        xt[:L_IN, :T, :] = xc.transpose(2, 1, 0)
        xt[L_IN, :T, :] = 1.0
        xt = xt.reshape(KX, xt_ticks * BC)
        xth, xtl = _bf16_split(xt)
        xstk = np.concatenate([xth, xtl, xth], axis=0)  # [93, cols]
        in_maps.append({
            "xts": xstk, "wes": wes, "fch": fch,
            "brs": brs, "ones": ones,
        })
    return in_maps


def _install_trace_hook():
    """Wire up the axon NTFF profiling hook (absent from this image)."""
    import types

    if "antenv.axon_hooks" in sys.modules:
        return True
    try:
        if "/root/.axon_site" not in sys.path:
            sys.path.insert(0, "/root/.axon_site")
        from trn_agent_boot.trn_boot import _ntff_profile_via_ctypes

        hook = _ntff_profile_via_ctypes("/opt/axon/libaxon_pjrt.so")
        if hook is None:
            return False
        mod = types.ModuleType("antenv.axon_hooks")
        mod.get_axon_ntff_profile_hook = lambda: hook
        sys.modules["antenv.axon_hooks"] = mod
        import concourse.bass_utils as bu

        bu.upload_artifacts = lambda tmpdir: str(tmpdir)
        return True
    except Exception as e:  # profiling is optional
        print(f"trace hook install failed: {e}", file=sys.stderr)
        return False


def run_cores(x, conv_w, conv_b, fc_w, fc_b, T=None):
    """Run the Bass kernel on len(batch)/32 cores; returns [B, 35] output."""
    global LAST_RESULTS
    T = T if T is not None else x.shape[1]
    trace = TRACE and _install_trace_hook()
    nc = _build_nc(T)
    in_maps = _host_prep(x, conv_w, conv_b, fc_w, fc_b, T)
    res = run_bass_kernel_spmd(
        nc, in_maps, core_ids=list(range(len(in_maps))), trace=trace,
    )
    LAST_RESULTS = res
    outs = []
    for i in range(len(in_maps)):
        hv = np.asarray(res.results[i]["hist"], dtype=np.float32)
        m2 = hv.reshape(J, T, BC)                  # [J, t, sample]
        outs.append((m2.sum(axis=1) / np.float32(T)).T.astype(np.float32))
    return np.concatenate(outs, axis=0)


def kernel(x, conv_w, conv_b, fc_w, fc_b):
    return run_cores(
        np.asarray(x, np.float32), np.asarray(conv_w, np.float32),
        np.asarray(conv_b, np.float32), np.asarray(fc_w, np.float32),
        np.asarray(fc_b, np.float32),
    )


# revision 17
# speedup vs baseline: 4.9041x; 1.0022x over previous
"""Trainium2 Bass kernel for nn_BasicClassifier (spiking conv classifier).

Sharding: pure data parallelism — batch 256 is split 32 samples per core
across 8 NeuronCores; params are replicated (tiny).

Per-core design. The T=1000 LIF scan is sequential, so the serial chain of
per-tick DVE ops paces everything. v2 cuts the per-op cost ~2.3x vs the
fp32/PSUM baseline by running the fused LIF op in the DVE's 2X_1PORT perf
mode: all operands are fp16, packed stride-1 in SBUF, and the op carries a
hand-written 2x uop program (element 0 in pipeline stages 0-3, element 1 in
stages 4-7 via the SRC_*_HI ports, results on WR0_LO/WR0_HI).

Memory plan per 16-tick block:
  - PSUM C tile [128, 4*512] fp32: banks 0-2 conv (bf16 hi/lo K-stacked
    GEMMs, ones row folds conv_b), bank 3 = fc bias prefill + fc result.
  - GpSimd copies the whole C tile to a tick-major fp16 SBUF tile (the DVE
    reads 128-col contiguous fp16 slices — 2x eligible).
  - state ring: fp16 [128, 16*128] x2. Per tick ONE fused DVE op:
    m' = (m*0.9 + c) - (m > 1).
  - spikes: ACT Sign over the fp16 ring -> sigma bf16; fc is a single-bf16
    halved-weight GEMM (3 matmuls) into bank 3 of the C tile 3 blocks ahead
    (the deep skew frees the copy from same-block dependencies).
  - mem2 history: fp16 DMA from ring cols 96:128 to DRAM; host sums.
"""

import os
import sys

for _p in ("/opt/trn_rl_repo", "/opt/pypackages"):
    if _p not in sys.path:
        sys.path.insert(0, _p)

import numpy as np

import concourse.bacc as bacc
import concourse.mybir as mybir
import concourse.tile as tile
import concourse.dve_ops as dve_ops
from concourse.dve_spec import Spec, Src0, Src1, C0, C1, lower
from concourse.dve_uop import (
    DveOpSpec, UopConfig, UopDpConfig, AluOp, AluInp, InpSel, OutSel, OutPath,
    DelayInp, Trigger, ENABLE, DISABLE, N_STAGES,
)
from concourse.bass_utils import run_bass_kernel_spmd

F32 = mybir.dt.float32
F16 = mybir.dt.float16
BF16 = mybir.dt.bfloat16
ALU = mybir.AluOpType
AF = mybir.ActivationFunctionType
AX = mybir.AxisListType

N_CORES = 8
B_FULL, T_FULL, L_IN = 256, 1000, 30
BC = B_FULL // N_CORES      # 32 samples per core
CH, LO = 16, 24
F = CH * LO                 # 384 features
G = 3                       # feature groups of 128
J = 35                      # fc outputs
KX = L_IN + 1               # conv contraction rows (30 taps + ones row)
BLK = 16                    # ticks per block (N = 16*32 = 512 = 1 PSUM bank)
LEAD = 2                    # C-tile pipeline lead in blocks (PSUM = 2 live tiles)
SKEW = LEAD * BLK           # layer-2 lag: spikes at tick t drive m2 at t+SKEW
WIN = 160                   # ticks per x-window DMA (multiple of BLK)
BETA, THR = 0.9, 1.0

TRACE = bool(int(os.environ.get("KERNEL_TRACE", "0")))
PERF_MAX = int(os.environ.get("KERNEL_PERF", "3"))
LAST_RESULTS = None

_LIF_OP = None

PREV = AluInp.PREV_ALU_OUT
SRC_DONE = (Trigger.SRC_TENSOR_DONE, Trigger.NONE, Trigger.NONE)


def _lif_2x_uop():
    """Hand-written 2X_1PORT program for m' = (m*C0 + c) - (m > C1).

    Per cycle the engine feeds two consecutive fp16 elements (one 32-bit
    word): SRC_0/SRC_1 = element 0, SRC_0_HI/SRC_1_HI = element 1. Stages
    0-3 compute element 0, stages 4-7 element 1; results ride delay lane 0
    (lo) and the final ALU out (hi).

    lanes: 0=SRC_0, 1=C0, 2=SRC_1, 3=C1, 4=SRC_0_HI, 5=SRC_1_HI
    """
    n = N_STAGES["v3"]
    dp = [UopDpConfig() for _ in range(n)]
    live = [
        (0, 1, 2, 3, 4, 5),   # s0
        (0, 1, 2, 3, 4, 5),   # s1
        (1, 3, 4, 5),         # s2 (lane0 captured here)
        (0, 1, 3, 4, 5),      # s3
        (3, 4, 5),            # s4 (lane0 captured here)
        (0, 3, 4, 5),         # s5
        (0, 3),               # s6 (lane4 captured here)
        (0, 4),               # s7
    ]
    for st in range(n):
        dp[st].pass_through_delay(*live[st])
    D = lambda i: AluInp(int(AluInp.PREV_DELAY_0) + i)
    dp[0].enable_alu(AluOp.MULTIPLY, D(0), D(1))          # m_lo * beta
    dp[1].enable_alu(AluOp.ADD, PREV, D(2))               # + c_lo
    dp[2].enable_alu(AluOp.IS_LT, D(3), D(0))             # thr < m_lo
    dp[2].enable_delay_from_src(DelayInp.PREV_ALU_OUT, 0)  # lane0 <- t2_lo
    dp[3].enable_alu(AluOp.SUBTRACT, D(0), PREV)          # res_lo
    dp[4].enable_alu(AluOp.MULTIPLY, D(4), D(1))          # m_hi * beta
    dp[4].enable_delay_from_src(DelayInp.PREV_ALU_OUT, 0)  # lane0 <- res_lo
    dp[5].enable_alu(AluOp.ADD, PREV, D(5))               # + c_hi
    dp[6].enable_alu(AluOp.IS_LT, D(3), D(4))             # thr < m_hi
    dp[6].enable_delay_from_src(DelayInp.PREV_ALU_OUT, 4)  # lane4 <- t2_hi
    dp[7].enable_alu(AluOp.SUBTRACT, D(4), PREV)          # res_hi

    inp = [InpSel.ZERO] * 8
    inp_enable = [DISABLE] * 8
    for slot, sel in ((1, InpSel.SRC_0), (2, InpSel.CONST_0), (3, InpSel.SRC_1),
                      (4, InpSel.CONST_1), (5, InpSel.SRC_0_HI),
                      (6, InpSel.SRC_1_HI)):
        inp[slot], inp_enable[slot] = sel, ENABLE
    out = {o: OutSel.ALU_OUT for o in OutPath}
    out_enable = {o: DISABLE for o in OutPath}
    out[OutPath.WR0_LO], out_enable[OutPath.WR0_LO] = OutSel.DELAY_0, ENABLE
    out[OutPath.WR0_HI], out_enable[OutPath.WR0_HI] = OutSel.ALU_OUT, ENABLE
    return UopConfig(
        datapath_config=dp,
        inp=inp,
        inp_enable=inp_enable,
        out=out,
        out_enable=out_enable,
        accum_enabled=DISABLE,
        require_inp0=1,
        require_inp1=1,
        trigger=SRC_DONE,
        next_uop=(0, 0, 0),
        repeat_count=0,
    )


def _get_lif_op():
    """Register the fused LIF-step op (with a 2x perf program) in the
    custom-DVE table (idempotent)."""
    global _LIF_OP
    if _LIF_OP is not None:
        return _LIF_OP
    name = "LIF_STEP_ANT59"
    for op in dve_ops.OPS:
        if op.name == name:
            _LIF_OP = op
            return op
    spec = Spec(
        body=(Src0 * C0 + Src1) - (Src0 > C1),
        reference=lambda in0, in1, s0, s1, imm2: (
            (in0.astype(np.float32) * np.float32(s0)
             + in1.reshape(in0.shape))
            - (in0 > s1).astype(np.float32)
        ).astype(np.float32),
    )
    row = dve_ops._CUSTOM_DVE_ROW_BASE + len(dve_ops.OPS)
    assert row < 0x20
    dve_ops._SUB_OPCODE_FOR_NAME[name] = row
    compiled = DveOpSpec(
        name=name, opcode=row, uops=lower(spec, ver="v3"), rd1_en=True,
        uops_2x=[_lif_2x_uop()], perf_max=1,
    )
    compiled.validate("v3")
    shas = {"v3": compiled.sha("v3")}
    op = dve_ops.DveOp(name, spec, subdim=False, uops_sha=shas)
    dve_ops.OPS.append(op)
    dve_ops.CUSTOM_DVE_SPECS[name] = spec
    # compile() path must return the spec WITH the 2x program: pre-seed the
    # memo that both _custom_dve and dve_table_for_ops consult.
    dve_ops._COMPILE_CACHE[(name, "v3")] = compiled
    _LIF_OP = op
    return op


def _build_nc(T):
    """Build the per-core Bass program (SPMD: same program on every core)."""
    lif = _get_lif_op()
    ticks = T + SKEW                       # DVE ticks 0..T+SKEW-1
    nblk = -(-ticks // BLK)
    pad_ticks = nblk * BLK
    windows = -(-pad_ticks // WIN)
    xt_cols = windows * WIN * BC
    NB = BLK * BC                          # 512: one PSUM bank of f32

    nc = bacc.Bacc("TRN2", target_bir_lowering=False)

    KS = 3 * KX                            # stacked conv K: [xh; xl; xh]
    xts_d = nc.dram_tensor("xts", [KS, xt_cols], BF16, kind="ExternalInput")
    wes_d = nc.dram_tensor("wes", [KS, F], BF16, kind="ExternalInput")
    fch_d = nc.dram_tensor("fch", [128, G * J], BF16, kind="ExternalInput")
    brs_d = nc.dram_tensor("brs", [2, 128], BF16, kind="ExternalInput")
    ones_d = nc.dram_tensor("ones", [2, NB], BF16, kind="ExternalInput")
    hist_d = nc.dram_tensor("hist", [J, BC * T], F16, kind="ExternalOutput")

    with tile.TileContext(nc) as tc:
        with (
            tc.tile_pool(name="konst", bufs=1) as kp,
            tc.tile_pool(name="ring", bufs=1) as rp,
            tc.tile_pool(name="sig", bufs=2) as sgp,
            tc.tile_pool(name="xwin", bufs=2) as xp,
            tc.tile_pool(name="csb", bufs=3) as cbp,
            tc.tile_pool(name="cpsum", bufs=2, space="PSUM") as cp,
        ):
            # constants -> SBUF
            wes = kp.tile([KS, F], BF16, tag="wes")
            fch = kp.tile([128, G * J], BF16, tag="fch")
            brs = kp.tile([2, 128], BF16, tag="brs")
            ones = kp.tile([2, NB], BF16, tag="ones")
            negthr = kp.tile([128, 1], F32, tag="negthr")
            nc.vector.memset(negthr[:], -THR)
            for sb, dr in ((wes, wes_d), (fch, fch_d),
                           (brs, brs_d), (ones, ones_d)):
                nc.sync.dma_start(sb[:], dr[:])

            # state ring: 2 block-sized fp16 tiles of 16 slices each
            ringA = rp.tile([128, BLK * 128], F16, tag="ringA")
            ringB = rp.tile([128, BLK * 128], F16, tag="ringB")
            nc.vector.memset(ringA[:], 0.0)
            nc.vector.memset(ringB[:], 0.0)
            rings = (ringA, ringB)

            xts = {}      # window idx -> xt sbuf tile
            chs = {}      # block idx -> PSUM C tile [128, 4*512] fp32
            csbs = {}     # block idx -> SBUF fp16 C tile [128, 16*128]
            def load_window(w):
                if w < 0 or w >= windows or w in xts:
                    return
                ts = xp.tile([KS, WIN * BC], BF16, tag="xws")
                nc.sync.dma_start(ts[:], xts_d[:, w * WIN * BC:(w + 1) * WIN * BC])
                xts[w] = ts

            def ensure_psum(b):
                """Allocate block b's PSUM C tile; conv-fill banks 0-2 and
                prime bank 3 with the fc bias (zeros for b < LEAD)."""
                if b >= nblk or b in chs:
                    return
                ch = cp.tile([128, 4 * NB], F32, tag="ch")
                chs[b] = ch
                w = (b * BLK) // WIN
                base = (b * BLK - w * WIN) * BC
                if b >= LEAD:
                    nc.tensor.matmul(
                        out=ch[:, G * NB:4 * NB],
                        lhsT=brs[:, :], rhs=ones[:, :],
                        start=True, stop=False,
                        skip_group_check=True,
                    )
                else:
                    nc.vector.memset(ch[:, G * NB:4 * NB], 0.0)
                for g in range(G):
                    nc.tensor.matmul(
                        out=ch[:, g * NB:(g + 1) * NB],
                        lhsT=wes[:, g * 128:(g + 1) * 128],
                        rhs=xts[w][:, base:base + NB],
                        start=True, stop=True,
                    )

            def conv_copy(b):
                """PSUM banks 0-2 of C tile b -> tick-major fp16 SBUF tile
                (ACT; GpSimd cannot read PSUM). psum col = g*512 + t*32 + c ;
                fp16 col = t*128 + g*32 + c. No fc dependency: runs early."""
                if b >= nblk or b in csbs:
                    return
                cs = cbp.tile([128, BLK * 128], F16, tag="cs")
                csbs[b] = cs
                src = chs[b][:, 0:G * NB].rearrange("p (g t c) -> p t g c", g=G, c=BC)
                dst = cs[:].rearrange("p (t g c) -> p t g c", g=4, c=BC)[:, :, 0:G, :]
                nc.scalar.activation(out=dst, in_=src, func=AF.Copy)

            def fc_copy(b):
                """PSUM bank 3 (fc drive) of C tile b -> fp16 cols 96:128
                (ACT; short chain after the same-block fc matmuls)."""
                if b >= nblk:
                    return
                src = chs[b][:, G * NB:4 * NB].rearrange("p (t c) -> p t c", c=BC)
                dst = csbs[b][:].rearrange("p (t g c) -> p t g c", g=4, c=BC)[:, :, G, :]
                nc.scalar.activation(out=dst, in_=src, func=AF.Copy)

            def spikes_and_fc(b):
                """After block b's ticks: sigma = Sign(m1 - 1) in {-1,0,1}
                (one ACT op over the fp16 ring; halved fc weights fold the
                (sigma+1)/2), then fc (3 bf16 matmuls) into C tile b+LEAD."""
                if b < 0 or b + LEAD >= nblk:
                    return
                ring4 = rings[b % 2][:].rearrange("p (t g c) -> p t g c", g=4, c=BC)
                sg = sgp.tile([128, G * NB], BF16, tag="sg")
                sg4 = sg[:].rearrange("p (g t c) -> p t g c", g=G, c=BC)
                nc.scalar.activation(
                    out=sg4, in_=ring4[:, :, 0:G, :],
                    func=AF.Sign, bias=negthr[:],
                )
                for g in range(G):
                    nc.tensor.matmul(
                        out=chs[b + LEAD][0:J, G * NB:4 * NB],
                        lhsT=fch[:, g * J:(g + 1) * J],
                        rhs=sg[:, g * NB:(g + 1) * NB],
                        start=False, stop=(g == G - 1),
                        skip_group_check=True,
                    )

            def hist_dma(b):
                """mem2 of DVE-tick block b = m2 ticks [16b-SKEW, ...):
                DMA straight from the fp16 ring to DRAM (host sums)."""
                t0 = b * BLK - SKEW
                if t0 < 0:
                    return
                n = min(BLK, T - t0)
                if n <= 0:
                    return
                ring3 = rings[b % 2][:].rearrange("p (t c) -> p t c", c=128)
                nc.sync.dma_start(
                    hist_d[:, t0 * BC:(t0 + n) * BC],
                    ring3[0:J, 0:n, G * BC:128],
                )

            # prologue: C pipeline primed LEAD blocks deep
            load_window(0)
            load_window(1)
            ensure_psum(0)
            ensure_psum(1)
            conv_copy(0)
            fc_copy(0)

            for t in range(ticks):
                b, lo = divmod(t, BLK)
                if lo == 0:
                    load_window((b * BLK) // WIN + 2)
                    ensure_psum(b + LEAD)
                    conv_copy(b + 1)
                    spikes_and_fc(b - 1)
                    fc_copy(b + 1)
                    hist_dma(b - 1)
                ring = rings[b % 2]
                prev = rings[(b - 1) % 2] if lo == 0 else ring
                plo = (lo - 1) % BLK

                inst = nc.vector._custom_dve(
                    lif,
                    out=ring[:, lo * 128:(lo + 1) * 128],
                    in0=prev[:, plo * 128:(plo + 1) * 128],
                    in1=csbs[b][:, lo * 128:(lo + 1) * 128],
                    s0=BETA, s1=THR,
                )
                inst.ins.perf_max = PERF_MAX

            # epilogue: the last block's mem2 history
            hist_dma(nblk - 1)

    nc.compile()
    return nc


def _bf16_split(a):
    import ml_dtypes
    hi = a.astype(ml_dtypes.bfloat16)
    lo = (a - hi.astype(np.float32)).astype(ml_dtypes.bfloat16)
    return hi, lo


def _host_prep(x, conv_w, conv_b, fc_w, fc_b, T):
    """Build per-core input maps (numpy only)."""
    import ml_dtypes
    ticks = T + SKEW
    nblk = -(-ticks // BLK)
    windows = -(-(nblk * BLK) // WIN)
    xt_ticks = windows * WIN

    wexp = np.zeros((KX, F), np.float32)
    for c in range(CH):
        for l in range(LO):
            wexp[l:l + 7, c * LO + l] = conv_w[c, 0, :]
        wexp[L_IN, c * LO:(c + 1) * LO] = conv_b[c]
    weh, wel = _bf16_split(wexp)
    wes = np.concatenate([weh, weh, wel], axis=0)  # K-stacked [93, F]

    # spike trick: s = (sigma+1)/2 with sigma = sign(m-1) in {-1,0,1}
    # c2 = fc_w @ s + b = (fc_w/2) @ sigma + (b + fc_w.sum/2)
    half = (fc_w * 0.5).astype(np.float32)
    fcwt = np.zeros((128, G * J), np.float32)
    for g in range(G):
        fcwt[:, g * J:(g + 1) * J] = half[:, g * 128:(g + 1) * 128].T
    fch = fcwt.astype(ml_dtypes.bfloat16)
    brow = np.zeros((1, 128), np.float32)
    brow[0, :J] = fc_b + half.sum(axis=1)
    brh, brl = _bf16_split(brow)
    brs = np.concatenate([brh, brl], axis=0)       # [2, 128]

    ones = np.ones((2, BLK * BC), ml_dtypes.bfloat16)

    in_maps = []
    B = x.shape[0]
    n_cores = B // BC
    for core in range(n_cores):
        xc = x[core * BC:(core + 1) * BC]          # [BC, T, L]
        xt = np.zeros((KX, xt_ticks, BC), np.float32)
        xt[:L_IN, :T, :] = xc.transpose(2, 1, 0)
        xt[L_IN, :T, :] = 1.0
        xt = xt.reshape(KX, xt_ticks * BC)
        xth, xtl = _bf16_split(xt)
        xstk = np.concatenate([xth, xtl, xth], axis=0)  # [93, cols]
        in_maps.append({
            "xts": xstk, "wes": wes, "fch": fch,
            "brs": brs, "ones": ones,
        })
    return in_maps


def _install_trace_hook():
    """Wire up the axon NTFF profiling hook (absent from this image)."""
    import types

    if "antenv.axon_hooks" in sys.modules:
        return True
    try:
        if "/root/.axon_site" not in sys.path:
            sys.path.insert(0, "/root/.axon_site")
        from trn_agent_boot.trn_boot import _ntff_profile_via_ctypes

        hook = _ntff_profile_via_ctypes("/opt/axon/libaxon_pjrt.so")
        if hook is None:
            return False
        mod = types.ModuleType("antenv.axon_hooks")
        mod.get_axon_ntff_profile_hook = lambda: hook
        sys.modules["antenv.axon_hooks"] = mod
        import concourse.bass_utils as bu

        bu.upload_artifacts = lambda tmpdir: str(tmpdir)
        return True
    except Exception as e:  # profiling is optional
        print(f"trace hook install failed: {e}", file=sys.stderr)
        return False


def run_cores(x, conv_w, conv_b, fc_w, fc_b, T=None):
    """Run the Bass kernel on len(batch)/32 cores; returns [B, 35] output."""
    global LAST_RESULTS
    T = T if T is not None else x.shape[1]
    trace = TRACE and _install_trace_hook()
    nc = _build_nc(T)
    in_maps = _host_prep(x, conv_w, conv_b, fc_w, fc_b, T)
    res = run_bass_kernel_spmd(
        nc, in_maps, core_ids=list(range(len(in_maps))), trace=trace,
    )
    LAST_RESULTS = res
    outs = []
    for i in range(len(in_maps)):
        hv = np.asarray(res.results[i]["hist"], dtype=np.float32)
        m2 = hv.reshape(J, T, BC)                  # [J, t, sample]
        outs.append((m2.sum(axis=1) / np.float32(T)).T.astype(np.float32))
    return np.concatenate(outs, axis=0)


def kernel(x, conv_w, conv_b, fc_w, fc_b):
    return run_cores(
        np.asarray(x, np.float32), np.asarray(conv_w, np.float32),
        np.asarray(conv_b, np.float32), np.asarray(fc_w, np.float32),
        np.asarray(fc_b, np.float32),
    )


# revision 21
# speedup vs baseline: 5.6127x; 1.1445x over previous
"""Trainium2 Bass kernel for nn_BasicClassifier (spiking conv classifier).

Sharding: pure data parallelism — batch 256 is split 32 samples per core
across 8 NeuronCores; params are replicated (tiny).

Per-core design. The T=1000 LIF scan is sequential, so the serial chain of
per-tick DVE ops paces everything. v2 cuts the per-op cost ~2.3x vs the
fp32/PSUM baseline by running the fused LIF op in the DVE's 2X_1PORT perf
mode: all operands are fp16, packed stride-1 in SBUF, and the op carries a
hand-written 2x uop program (element 0 in pipeline stages 0-3, element 1 in
stages 4-7 via the SRC_*_HI ports, results on WR0_LO/WR0_HI).

Memory plan per 16-tick block:
  - PSUM C tile [128, 4*512] fp32: banks 0-2 conv (bf16 hi/lo K-stacked
    GEMMs, ones row folds conv_b), bank 3 = fc bias prefill + fc result.
  - GpSimd copies the whole C tile to a tick-major fp16 SBUF tile (the DVE
    reads 128-col contiguous fp16 slices — 2x eligible).
  - state ring: fp16 [128, 16*128] x2. Per tick ONE fused DVE op:
    m' = (m*0.9 + c) - (m > 1).
  - spikes: ACT Sign over the fp16 ring -> sigma bf16; fc is a single-bf16
    halved-weight GEMM (3 matmuls) into bank 3 of the C tile 3 blocks ahead
    (the deep skew frees the copy from same-block dependencies).
  - mem2 history: fp16 DMA from ring cols 96:128 to DRAM; host sums.
"""

import os
import sys

for _p in ("/opt/trn_rl_repo", "/opt/pypackages"):
    if _p not in sys.path:
        sys.path.insert(0, _p)

import numpy as np

import concourse.bacc as bacc
import concourse.mybir as mybir
import concourse.tile as tile
import concourse.dve_ops as dve_ops
from concourse.dve_spec import Spec, Src0, Src1, C0, C1, lower
from concourse.dve_uop import (
    DveOpSpec, UopConfig, UopDpConfig, AluOp, AluInp, InpSel, OutSel, OutPath,
    DelayInp, Trigger, ENABLE, DISABLE, N_STAGES,
)
from concourse.bass_utils import run_bass_kernel_spmd

F32 = mybir.dt.float32
F16 = mybir.dt.float16
BF16 = mybir.dt.bfloat16
ALU = mybir.AluOpType
AF = mybir.ActivationFunctionType
AX = mybir.AxisListType

N_CORES = 8
B_FULL, T_FULL, L_IN = 256, 1000, 30
BC = B_FULL // N_CORES      # 32 samples per core
CH, LO = 16, 24
F = CH * LO                 # 384 features
G = 3                       # feature groups of 128
J = 35                      # fc outputs
KX = L_IN + 1               # conv contraction rows (30 taps + ones row)
BLK = 16                    # ticks per block (N = 16*32 = 512 = 1 PSUM bank)
LEAD = 2                    # C-tile pipeline lead in blocks (PSUM = 2 live tiles)
SKEW = LEAD * BLK           # layer-2 lag: spikes at tick t drive m2 at t+SKEW
WIN = 160                   # ticks per x-window DMA (multiple of BLK)
BETA, THR = 0.9, 1.0

TRACE = bool(int(os.environ.get("KERNEL_TRACE", "0")))
PERF_MAX = int(os.environ.get("KERNEL_PERF", "3"))
LAST_RESULTS = None

_LIF_OP = None

PREV = AluInp.PREV_ALU_OUT
SRC_DONE = (Trigger.SRC_TENSOR_DONE, Trigger.NONE, Trigger.NONE)


def _lif_2x_uop():
    """Hand-written 2X_1PORT program for m' = (m*C0 + c) - (m > C1).

    Per cycle the engine feeds two consecutive fp16 elements (one 32-bit
    word): SRC_0/SRC_1 = element 0, SRC_0_HI/SRC_1_HI = element 1. Stages
    0-3 compute element 0, stages 4-7 element 1; results ride delay lane 0
    (lo) and the final ALU out (hi).

    lanes: 0=SRC_0, 1=C0, 2=SRC_1, 3=C1, 4=SRC_0_HI, 5=SRC_1_HI
    """
    n = N_STAGES["v3"]
    dp = [UopDpConfig() for _ in range(n)]
    live = [
        (0, 1, 2, 3, 4, 5),   # s0
        (0, 1, 2, 3, 4, 5),   # s1
        (1, 3, 4, 5),         # s2 (lane0 captured here)
        (0, 1, 3, 4, 5),      # s3
        (3, 4, 5),            # s4 (lane0 captured here)
        (0, 3, 4, 5),         # s5
        (0, 3),               # s6 (lane4 captured here)
        (0, 4),               # s7
    ]
    for st in range(n):
        dp[st].pass_through_delay(*live[st])
    D = lambda i: AluInp(int(AluInp.PREV_DELAY_0) + i)
    dp[0].enable_alu(AluOp.MULTIPLY, D(0), D(1))          # m_lo * beta
    dp[1].enable_alu(AluOp.ADD, PREV, D(2))               # + c_lo
    dp[2].enable_alu(AluOp.IS_LT, D(3), D(0))             # thr < m_lo
    dp[2].enable_delay_from_src(DelayInp.PREV_ALU_OUT, 0)  # lane0 <- t2_lo
    dp[3].enable_alu(AluOp.SUBTRACT, D(0), PREV)          # res_lo
    dp[4].enable_alu(AluOp.MULTIPLY, D(4), D(1))          # m_hi * beta
    dp[4].enable_delay_from_src(DelayInp.PREV_ALU_OUT, 0)  # lane0 <- res_lo
    dp[5].enable_alu(AluOp.ADD, PREV, D(5))               # + c_hi
    dp[6].enable_alu(AluOp.IS_LT, D(3), D(4))             # thr < m_hi
    dp[6].enable_delay_from_src(DelayInp.PREV_ALU_OUT, 4)  # lane4 <- t2_hi
    dp[7].enable_alu(AluOp.SUBTRACT, D(4), PREV)          # res_hi

    inp = [InpSel.ZERO] * 8
    inp_enable = [DISABLE] * 8
    for slot, sel in ((1, InpSel.SRC_0), (2, InpSel.CONST_0), (3, InpSel.SRC_1),
                      (4, InpSel.CONST_1), (5, InpSel.SRC_0_HI),
                      (6, InpSel.SRC_1_HI)):
        inp[slot], inp_enable[slot] = sel, ENABLE
    out = {o: OutSel.ALU_OUT for o in OutPath}
    out_enable = {o: DISABLE for o in OutPath}
    out[OutPath.WR0_LO], out_enable[OutPath.WR0_LO] = OutSel.DELAY_0, ENABLE
    out[OutPath.WR0_HI], out_enable[OutPath.WR0_HI] = OutSel.ALU_OUT, ENABLE
    return UopConfig(
        datapath_config=dp,
        inp=inp,
        inp_enable=inp_enable,
        out=out,
        out_enable=out_enable,
        accum_enabled=DISABLE,
        require_inp0=1,
        require_inp1=1,
        trigger=SRC_DONE,
        next_uop=(0, 0, 0),
        repeat_count=0,
    )


def _get_lif_op():
    """Register the fused LIF-step op (with a 2x perf program) in the
    custom-DVE table (idempotent)."""
    global _LIF_OP
    if _LIF_OP is not None:
        return _LIF_OP
    name = "LIF_STEP_ANT59"
    for op in dve_ops.OPS:
        if op.name == name:
            _LIF_OP = op
            return op
    spec = Spec(
        body=(Src0 * C0 + Src1) - (Src0 > C1),
        reference=lambda in0, in1, s0, s1, imm2: (
            (in0.astype(np.float32) * np.float32(s0)
             + in1.reshape(in0.shape))
            - (in0 > s1).astype(np.float32)
        ).astype(np.float32),
    )
    row = dve_ops._CUSTOM_DVE_ROW_BASE + len(dve_ops.OPS)
    assert row < 0x20
    dve_ops._SUB_OPCODE_FOR_NAME[name] = row
    compiled = DveOpSpec(
        name=name, opcode=row, uops=lower(spec, ver="v3"), rd1_en=True,
        uops_2x=[_lif_2x_uop()], perf_max=1,
    )
    compiled.validate("v3")
    shas = {"v3": compiled.sha("v3")}
    op = dve_ops.DveOp(name, spec, subdim=False, uops_sha=shas)
    dve_ops.OPS.append(op)
    dve_ops.CUSTOM_DVE_SPECS[name] = spec
    # compile() path must return the spec WITH the 2x program: pre-seed the
    # memo that both _custom_dve and dve_table_for_ops consult.
    dve_ops._COMPILE_CACHE[(name, "v3")] = compiled
    _LIF_OP = op
    return op


def _build_nc(T):
    """Build the per-core Bass program (SPMD: same program on every core)."""
    lif = _get_lif_op()
    ticks = T + SKEW                       # DVE ticks 0..T+SKEW-1
    nblk = -(-ticks // BLK)
    pad_ticks = nblk * BLK
    windows = -(-pad_ticks // WIN)
    xt_cols = windows * WIN * BC
    NB = BLK * BC                          # 512: one PSUM bank of f32

    nc = bacc.Bacc("TRN2", target_bir_lowering=False)

    KS = 3 * KX                            # stacked conv K: [xh; xl; xh]
    xts_d = nc.dram_tensor("xts", [KS, xt_cols], BF16, kind="ExternalInput")
    wes_d = nc.dram_tensor("wes", [KS, F], BF16, kind="ExternalInput")
    fch_d = nc.dram_tensor("fch", [128, G * J], BF16, kind="ExternalInput")
    brs_d = nc.dram_tensor("brs", [2, 128], BF16, kind="ExternalInput")
    ones_d = nc.dram_tensor("ones", [2, NB], BF16, kind="ExternalInput")
    hist_d = nc.dram_tensor("hist", [J, BC * T], F16, kind="ExternalOutput")

    with tile.TileContext(nc) as tc:
        with (
            tc.tile_pool(name="konst", bufs=1) as kp,
            tc.tile_pool(name="ring", bufs=1) as rp,
            tc.tile_pool(name="sig", bufs=2) as sgp,
            tc.tile_pool(name="xwin", bufs=2) as xp,
            tc.tile_pool(name="csb", bufs=3) as cbp,
            tc.tile_pool(name="cpsum", bufs=2, space="PSUM") as cp,
        ):
            # constants -> SBUF
            wes = kp.tile([KS, F], BF16, tag="wes")
            fch = kp.tile([128, G * J], BF16, tag="fch")
            brs = kp.tile([2, 128], BF16, tag="brs")
            ones = kp.tile([2, NB], BF16, tag="ones")
            negthr = kp.tile([128, 1], F32, tag="negthr")
            nc.vector.memset(negthr[:], -THR)
            for sb, dr in ((wes, wes_d), (fch, fch_d),
                           (brs, brs_d), (ones, ones_d)):
                nc.sync.dma_start(sb[:], dr[:])

            # state ring: 3 block-sized fp16 tiles of 16 slices each (the
            # third buys WAR slack so late hist DMAs don't stall the DVE)
            ringA = rp.tile([128, BLK * 128], F16, tag="ringA")
            ringB = rp.tile([128, BLK * 128], F16, tag="ringB")
            ringC = rp.tile([128, BLK * 128], F16, tag="ringC")
            rings = (ringA, ringB, ringC)
            for r in rings:
                nc.vector.memset(r[:], 0.0)
            NR = len(rings)

            xts = {}      # window idx -> xt sbuf tile
            chs = {}      # block idx -> PSUM C tile [128, 4*512] fp32
            csbs = {}     # block idx -> SBUF fp16 C tile [128, 16*128]
            def load_window(w):
                # chunked into 5 DMAs so latency-critical hist DMAs behind
                # them on the same queue wait ~2us, not the whole window
                if w < 0 or w >= windows or w in xts:
                    return
                ts = xp.tile([KS, WIN * BC], BF16, tag="xws")
                step = WIN * BC // 5
                for i in range(5):
                    nc.sync.dma_start(
                        ts[:, i * step:(i + 1) * step],
                        xts_d[:, w * WIN * BC + i * step:w * WIN * BC + (i + 1) * step],
                    )
                xts[w] = ts

            def ensure_psum(b):
                """Allocate block b's PSUM C tile; conv-fill banks 0-2 and
                prime bank 3 with the fc bias (zeros for b < LEAD)."""
                if b >= nblk or b in chs:
                    return
                ch = cp.tile([128, 4 * NB], F32, tag="ch")
                chs[b] = ch
                w = (b * BLK) // WIN
                base = (b * BLK - w * WIN) * BC
                if b >= LEAD:
                    nc.tensor.matmul(
                        out=ch[:, G * NB:4 * NB],
                        lhsT=brs[:, :], rhs=ones[:, :],
                        start=True, stop=False,
                        skip_group_check=True,
                    )
                else:
                    nc.vector.memset(ch[:, G * NB:4 * NB], 0.0)
                for g in range(G):
                    nc.tensor.matmul(
                        out=ch[:, g * NB:(g + 1) * NB],
                        lhsT=wes[:, g * 128:(g + 1) * 128],
                        rhs=xts[w][:, base:base + NB],
                        start=True, stop=True,
                    )

            def conv_copy(b):
                """PSUM banks 0-2 of C tile b -> tick-major fp16 SBUF tile
                (ACT; GpSimd cannot read PSUM). psum col = g*512 + t*32 + c ;
                fp16 col = t*128 + g*32 + c. No fc dependency: runs early."""
                if b >= nblk or b in csbs:
                    return
                cs = cbp.tile([128, BLK * 128], F16, tag="cs")
                csbs[b] = cs
                src = chs[b][:, 0:G * NB].rearrange("p (g t c) -> p t g c", g=G, c=BC)
                dst = cs[:].rearrange("p (t g c) -> p t g c", g=4, c=BC)[:, :, 0:G, :]
                nc.scalar.activation(out=dst, in_=src, func=AF.Copy)

            def fc_copy(b):
                """PSUM bank 3 (fc drive) of C tile b -> fp16 cols 96:128
                (ACT; short chain after the same-block fc matmuls)."""
                if b >= nblk:
                    return
                src = chs[b][:, G * NB:4 * NB].rearrange("p (t c) -> p t c", c=BC)
                dst = csbs[b][:].rearrange("p (t g c) -> p t g c", g=4, c=BC)[:, :, G, :]
                nc.scalar.activation(out=dst, in_=src, func=AF.Copy)

            def spikes_and_fc(b):
                """After block b's ticks: sigma = Sign(m1 - 1) in {-1,0,1}
                (one ACT op over the fp16 ring; halved fc weights fold the
                (sigma+1)/2), then fc (3 bf16 matmuls) into C tile b+LEAD."""
                if b < 0 or b + LEAD >= nblk:
                    return
                ring4 = rings[b % NR][:].rearrange("p (t g c) -> p t g c", g=4, c=BC)
                sg = sgp.tile([128, G * NB], BF16, tag="sg")
                sg4 = sg[:].rearrange("p (g t c) -> p t g c", g=G, c=BC)
                nc.scalar.activation(
                    out=sg4, in_=ring4[:, :, 0:G, :],
                    func=AF.Sign, bias=negthr[:],
                )
                for g in range(G):
                    nc.tensor.matmul(
                        out=chs[b + LEAD][0:J, G * NB:4 * NB],
                        lhsT=fch[:, g * J:(g + 1) * J],
                        rhs=sg[:, g * NB:(g + 1) * NB],
                        start=False, stop=(g == G - 1),
                        skip_group_check=True,
                    )

            def hist_dma(b):
                """mem2 of DVE-tick block b = m2 ticks [16b-SKEW, ...):
                DMA straight from the fp16 ring to DRAM (host sums)."""
                t0 = b * BLK - SKEW
                if t0 < 0:
                    return
                n = min(BLK, T - t0)
                if n <= 0:
                    return
                ring3 = rings[b % NR][:].rearrange("p (t c) -> p t c", c=128)
                nc.sync.dma_start(
                    hist_d[:, t0 * BC:(t0 + n) * BC],
                    ring3[0:J, 0:n, G * BC:128],
                )

            # prologue: C pipeline primed LEAD blocks deep
            load_window(0)
            load_window(1)
            ensure_psum(0)
            ensure_psum(1)
            conv_copy(0)
            fc_copy(0)

            for t in range(ticks):
                b, lo = divmod(t, BLK)
                if lo == 0:
                    load_window((b * BLK) // WIN + 2)
                    ensure_psum(b + LEAD)
                    conv_copy(b + 1)
                    spikes_and_fc(b - 1)
                    fc_copy(b + 1)
                    hist_dma(b - 1)
                ring = rings[b % NR]
                prev = rings[(b - 1) % NR] if lo == 0 else ring
                plo = (lo - 1) % BLK

                inst = nc.vector._custom_dve(
                    lif,
                    out=ring[:, lo * 128:(lo + 1) * 128],
                    in0=prev[:, plo * 128:(plo + 1) * 128],
                    in1=csbs[b][:, lo * 128:(lo + 1) * 128],
                    s0=BETA, s1=THR,
                )
                inst.ins.perf_max = PERF_MAX

            # epilogue: the last block's mem2 history
            hist_dma(nblk - 1)

    nc.compile()
    return nc


def _bf16_split(a):
    import ml_dtypes
    hi = a.astype(ml_dtypes.bfloat16)
    lo = (a - hi.astype(np.float32)).astype(ml_dtypes.bfloat16)
    return hi, lo


def _host_prep(x, conv_w, conv_b, fc_w, fc_b, T):
    """Build per-core input maps (numpy only)."""
    import ml_dtypes
    ticks = T + SKEW
    nblk = -(-ticks // BLK)
    windows = -(-(nblk * BLK) // WIN)
    xt_ticks = windows * WIN

    wexp = np.zeros((KX, F), np.float32)
    for c in range(CH):
        for l in range(LO):
            wexp[l:l + 7, c * LO + l] = conv_w[c, 0, :]
        wexp[L_IN, c * LO:(c + 1) * LO] = conv_b[c]
    weh, wel = _bf16_split(wexp)
    wes = np.concatenate([weh, weh, wel], axis=0)  # K-stacked [93, F]

    # spike trick: s = (sigma+1)/2 with sigma = sign(m-1) in {-1,0,1}
    # c2 = fc_w @ s + b = (fc_w/2) @ sigma + (b + fc_w.sum/2)
    half = (fc_w * 0.5).astype(np.float32)
    fcwt = np.zeros((128, G * J), np.float32)
    for g in range(G):
        fcwt[:, g * J:(g + 1) * J] = half[:, g * 128:(g + 1) * 128].T
    fch = fcwt.astype(ml_dtypes.bfloat16)
    brow = np.zeros((1, 128), np.float32)
    brow[0, :J] = fc_b + half.sum(axis=1)
    brh, brl = _bf16_split(brow)
    brs = np.concatenate([brh, brl], axis=0)       # [2, 128]

    ones = np.ones((2, BLK * BC), ml_dtypes.bfloat16)

    in_maps = []
    B = x.shape[0]
    n_cores = B // BC
    for core in range(n_cores):
        xc = x[core * BC:(core + 1) * BC]          # [BC, T, L]
        xt = np.zeros((KX, xt_ticks, BC), np.float32)
        xt[:L_IN, :T, :] = xc.transpose(2, 1, 0)
        xt[L_IN, :T, :] = 1.0
        xt = xt.reshape(KX, xt_ticks * BC)
        xth, xtl = _bf16_split(xt)
        xstk = np.concatenate([xth, xtl, xth], axis=0)  # [93, cols]
        in_maps.append({
            "xts": xstk, "wes": wes, "fch": fch,
            "brs": brs, "ones": ones,
        })
    return in_maps


def _install_trace_hook():
    """Wire up the axon NTFF profiling hook (absent from this image)."""
    import types

    if "antenv.axon_hooks" in sys.modules:
        return True
    try:
        if "/root/.axon_site" not in sys.path:
            sys.path.insert(0, "/root/.axon_site")
        from trn_agent_boot.trn_boot import _ntff_profile_via_ctypes

        hook = _ntff_profile_via_ctypes("/opt/axon/libaxon_pjrt.so")
        if hook is None:
            return False
        mod = types.ModuleType("antenv.axon_hooks")
        mod.get_axon_ntff_profile_hook = lambda: hook
        sys.modules["antenv.axon_hooks"] = mod
        import concourse.bass_utils as bu

        bu.upload_artifacts = lambda tmpdir: str(tmpdir)
        return True
    except Exception as e:  # profiling is optional
        print(f"trace hook install failed: {e}", file=sys.stderr)
        return False


def run_cores(x, conv_w, conv_b, fc_w, fc_b, T=None):
    """Run the Bass kernel on len(batch)/32 cores; returns [B, 35] output."""
    global LAST_RESULTS
    T = T if T is not None else x.shape[1]
    trace = TRACE and _install_trace_hook()
    nc = _build_nc(T)
    in_maps = _host_prep(x, conv_w, conv_b, fc_w, fc_b, T)
    res = run_bass_kernel_spmd(
        nc, in_maps, core_ids=list(range(len(in_maps))), trace=trace,
    )
    LAST_RESULTS = res
    outs = []
    for i in range(len(in_maps)):
        hv = np.asarray(res.results[i]["hist"], dtype=np.float32)
        m2 = hv.reshape(J, T, BC)                  # [J, t, sample]
        outs.append((m2.sum(axis=1) / np.float32(T)).T.astype(np.float32))
    return np.concatenate(outs, axis=0)


def kernel(x, conv_w, conv_b, fc_w, fc_b):
    return run_cores(
        np.asarray(x, np.float32), np.asarray(conv_w, np.float32),
        np.asarray(conv_b, np.float32), np.asarray(fc_w, np.float32),
        np.asarray(fc_b, np.float32),
    )
